# revision 23
# baseline (speedup 1.0000x reference)
"""KL-attention kernel for Trainium2, 8-core data-parallel over batch.

Math (per batch b, x = [N=1024, D=1024] fp32):
  p = softmax(x, -1); logp = log_softmax(x, -1)
  S[i,j] = sum_d p[i,d] logp[j,d]   (attn = softmax(S, -1); the neg-entropy
                                     row offset cancels in the row softmax)
  Using sum_d p[i,d] = 1:  S[i,j] = (p @ x^T)[i,j] - logZ[j]
  out = softmax(S, -1) @ x

fp8 implementation (all PE work is float8e4 DoubleRow matmuls, K=256 per
pass, 0.5 cycles/row in the cost model):
  E8 = fp8(exp(x)), row-accumulated -> Z                       (ACT)
  pt8 = fp8( E8^T * diag(rz*SC) )   via DR matmul vs paired block-diag
  xt8 = x8^T                        via DR matmul vs paired identity
  ps_s[j,i] = SC * sum_d x8[j,d] p[i,d]                        (MM1, DR)
  est_f = exp(ps_s / SC)            (ACT, scale immediate)
  est_hi = fp8(est_f * rzc_j); est_lo = fp8(est_f*rzc_j - est_hi)
    where rzc = rz * e^C carries the -logZ_j column bias multiplicatively
  U = hi@x8 + hi@x_lo + lo@x8 (DR); z = (hi+lo)@ones (DR)      (MM2)
  out = U * (1/z)                    (ACT copy with per-partition scale)

The est/x residual splits keep fp8 quantization error ~5e-3 (validated in
numpy; single-fp8 would be ~3e-2). Batches are software-pipelined: batch
b+1's exp/casts interleave with batch b's MM1, its transposes run between
MM1(b) and MM2(b) on the PE, and its psum copies interleave with MM2(b)'s
output scaling, so every engine sees a dense FIFO.
"""

import os

import numpy as np

try:
    import concourse.bass as bass  # noqa: F401
except ImportError:
    import sys

    sys.path.insert(0, "/opt/trn_rl_repo")

from contextlib import ExitStack

import concourse.bass as bass
import concourse.mybir as mybir
import concourse.tile as tile
from concourse import bacc
from concourse.bass_utils import run_bass_kernel_spmd
from concourse.masks import make_identity

F32 = mybir.dt.float32
BF16 = mybir.dt.bfloat16
F8 = mybir.dt.float8e4
AF = mybir.ActivationFunctionType
DR = mybir.MatmulPerfMode.DoubleRow
ALU = mybir.AluOpType

N_CORES = 8
B_PER_CORE = 4
N = 1024
D = 1024
P = 128
T = N // P  # 8 row tiles
TP = T // 2  # 4 tile pairs (DoubleRow K=256)
SC = 512.0  # p scaling so fp8(p*SC) stays in normal range
C = 7.43  # ~log(N*E[e^x]) rescale for est into fp8 range

# engine split for psum->sbuf fp8 copy passes (xt8, pt8): tiles 0..a-1 on
# ACT, the rest on DVE. (Pool/GPSIMD cannot read PSUM.)
XT_ACT = 3
PT_ACT = 2


class _Stages:
    """Per-batch stage emitters over shared pools; call order = schedule."""

    def __init__(self, nc, pools, consts, x_ap, out_ap):
        self.nc = nc
        self.p = pools
        self.c = consts
        self.x_ap = x_ap
        self.out_ap = out_ap
        self.s = [dict() for _ in range(B_PER_CORE)]

    def load(self, b):
        st = self.s[b]
        st["xf"] = self.p["xf"].tile([P, T, D], F32, tag="xf", name="xf")
        for t in range(T):
            self.nc.sync.dma_start(
                st["xf"][:, t, :], self.x_ap[b, t * P : (t + 1) * P, :]
            )

    def alloc_eb(self, b):
        st = self.s[b]
        st["eb8"] = self.p["eb"].tile([P, T, D], F8, tag="eb", name="eb8")
        st["zs"] = self.p["st"].tile([P, T], F32, tag="zs", name="zs")

    def exp_tile(self, b, t):
        st = self.s[b]
        self.nc.scalar.activation(
            st["eb8"][:, t, :],
            st["xf"][:, t, :],
            AF.Exp,
            accum_out=st["zs"][:, t : t + 1],
        )

    def stats(self, b):
        st = self.s[b]
        st["rz"] = self.p["st"].tile([P, T], F32, tag="rz", name="rz")
        st["rzsc"] = self.p["st"].tile([P, T], F32, tag="rzsc", name="rzsc")
        st["rzc"] = self.p["st"].tile([P, T], F32, tag="rzc", name="rzc")
        self.nc.vector.reciprocal(st["rz"][:, :], st["zs"][:, :])
        self.nc.vector.tensor_scalar_mul(st["rzsc"][:, :], st["rz"][:, :], SC)
        self.nc.vector.tensor_scalar_mul(
            st["rzc"][:, :], st["rz"][:, :], float(np.exp(C))
        )

    def dgp(self, b):
        st = self.s[b]
        st["dgp"] = self.c["dgp"]  # shared zeroed tile; diag blocks rewritten
        dgp, rzsc = st["dgp"], st["rzsc"]
        for m in range(TP):
            self.nc.gpsimd.tensor_scalar_mul(
                dgp[:, m, 0, 0:P], self.c["ident8"][:, :], rzsc[:, 2 * m : 2 * m + 1]
            )
            self.nc.gpsimd.tensor_scalar_mul(
                dgp[:, m, 1, P : 2 * P],
                self.c["ident8"][:, :],
                rzsc[:, 2 * m + 1 : 2 * m + 2],
            )

    def alloc_x8(self, b):
        st = self.s[b]
        st["x8"] = self.p["x8"].tile([P, T, D], F8, tag="x8", name="x8")
        st["xl8"] = self.p["xl"].tile([P, T, D], F8, tag="xl", name="xl8")

    def cast_x8(self, b, t):
        st = self.s[b]
        self.nc.gpsimd.tensor_copy(st["x8"][:, t, :], st["xf"][:, t, :])

    def cast_xl(self, b, t):
        st = self.s[b]
        self.nc.vector.scalar_tensor_tensor(
            st["xl8"][:, t, :],
            st["xf"][:, t, :],
            1.0,
            st["x8"][:, t, :],
            ALU.mult,
            ALU.subtract,
        )

    def xtT(self, b, ks):
        nc, st = self.nc, self.s[b]
        if "xt8" not in st:
            st["xt8"] = self.p["xt"].tile([P, T, D], F8, tag="xt", name="xt8")
        for k in ks:
            ps_x = self.p["mmps"].tile([P, D], F32, tag="ps", name="ps")
            for m in range(TP):
                nc.tensor.matmul(
                    ps_x[:, m * 2 * P : (m + 1) * 2 * P],
                    st["x8"][:, 2 * m : 2 * m + 2, k * P : (k + 1) * P],
                    self.c["ipair"][:, :, :],
                    start=True,
                    stop=True,
                    perf_mode=DR,
                )
            if k < 3:
                nc.scalar.copy(st["xt8"][:, k, :], ps_x[:, :])
            else:
                nc.vector.tensor_copy(st["xt8"][:, k, :], ps_x[:, :])

    def ptT(self, b, ks):
        nc, st = self.nc, self.s[b]
        if "pt8" not in st:
            st["pt8"] = self.p["pt"].tile([P, T, D], F8, tag="pt", name="pt8")
        for k in ks:
            ps_p = self.p["mmps"].tile([P, D], F32, tag="ps", name="ps")
            for m in range(TP):
                nc.tensor.matmul(
                    ps_p[:, m * 2 * P : (m + 1) * 2 * P],
                    st["eb8"][:, 2 * m : 2 * m + 2, k * P : (k + 1) * P],
                    st["dgp"][:, m, :, :],
                    start=True,
                    stop=True,
                    perf_mode=DR,
                )
            nc.scalar.copy(st["pt8"][:, k, :], ps_p[:, :])

    def alloc_est(self, b):
        st = self.s[b]
        st["eh8"] = self.p["eh"].tile([P, T, D], F8, tag="eh", name="eh8")
        st["el8"] = self.p["el"].tile([P, T, D], F8, tag="el", name="el8")

    def mm1_j(self, b, j):
        nc, st = self.nc, self.s[b]
        ps_s = self.p["mmps"].tile([P, D], F32, tag="ps", name="ps")
        for dp in range(TP):
            for c in range(2):
                nc.tensor.matmul(
                    ps_s[:, c * 512 : (c + 1) * 512],
                    st["xt8"][:, 2 * dp : 2 * dp + 2, j * P : (j + 1) * P],
                    st["pt8"][:, 2 * dp : 2 * dp + 2, c * 512 : (c + 1) * 512],
                    start=(dp == 0),
                    stop=(dp == TP - 1),
                    perf_mode=DR,
                )
        est_f = self.p["ef"].tile([P, D], F32, tag="ef", name="est_f")
        nc.scalar.activation(est_f[:, :], ps_s[:, :], AF.Exp, scale=1.0 / SC)
        if j < 6:
            nc.gpsimd.tensor_scalar_mul(
                st["eh8"][:, j, :], est_f[:, :], st["rzc"][:, j : j + 1]
            )
        else:
            nc.scalar.mul(st["eh8"][:, j, :], est_f[:, :], st["rzc"][:, j : j + 1])
        nc.vector.scalar_tensor_tensor(
            st["el8"][:, j, :],
            est_f[:, :],
            st["rzc"][:, j : j + 1],
            st["eh8"][:, j, :],
            ALU.mult,
            ALU.subtract,
        )

    def mm2_products(self, b, i):
        nc, st = self.nc, self.s[b]
        st[f"ps_o{i}"] = ps_o = self.p["mmps"].tile([P, D], F32, tag="ps", name="ps")
        prods = ((st["eh8"], st["x8"]), (st["eh8"], st["xl8"]), (st["el8"], st["x8"]))
        # jp outer so early j-pairs of est can feed the PE before the whole
        # est split finishes (range-precise deps let MM2 crawl with MM1)
        idx = 0
        for jp in range(TP):
            for lhs, rhs in prods:
                for c in range(2):
                    nc.tensor.matmul(
                        ps_o[:, c * 512 : (c + 1) * 512],
                        lhs[:, 2 * jp : 2 * jp + 2, i * P : (i + 1) * P],
                        rhs[:, 2 * jp : 2 * jp + 2, c * 512 : (c + 1) * 512],
                        start=(idx == 0),
                        stop=(idx == 3 * TP - 1),
                        perf_mode=DR,
                    )
                idx += 1

    def mm2_zout(self, b, i):
        nc, st = self.nc, self.s[b]
        ps_o = st.pop(f"ps_o{i}")
        ps_z = self.p["zps"].tile([P, 8], F32, tag="psz", name="psz")
        for jp in range(TP):
            for li, lhs in ((0, st["eh8"]), (1, st["el8"])):
                nc.tensor.matmul(
                    ps_z[:, 0:8],
                    lhs[:, 2 * jp : 2 * jp + 2, i * P : (i + 1) * P],
                    self.c["ones8"][:, :, :],
                    start=(jp == 0 and li == 0),
                    stop=(jp == TP - 1 and li == 1),
                    perf_mode=DR,
                )
        rzi = self.p["st"].tile([P, 1], F32, tag="rzi", name="rzi")
        nc.vector.reciprocal(rzi[:, :], ps_z[:, 0:1])
        outf = self.p["of"].tile([P, D], F32, tag="of", name="outf")
        if i < 3:
            nc.scalar.mul(outf[:, :], ps_o[:, :], rzi[:, :])
        else:
            nc.vector.tensor_scalar_mul(outf[:, :], ps_o[:, :], rzi[:, :])
        nc.sync.dma_start(self.out_ap[b, i * P : (i + 1) * P, :], outf[:, :])


def build_kernel_body(ctx: ExitStack, tc: "tile.TileContext", x_ap, out_ap):
    nc = tc.nc

    pools = {}
    for name, bufs, space in (
        ("xf", 2, "SBUF"),
        ("eb", 2, "SBUF"),
        ("x8", 2, "SBUF"),
        ("xl", 2, "SBUF"),
        ("xt", 1, "SBUF"),
        ("pt", 1, "SBUF"),
        ("ef", 3, "SBUF"),
        ("eh", 1, "SBUF"),
        ("el", 1, "SBUF"),
        ("of", 4, "SBUF"),
        ("st", 2, "SBUF"),
        ("mmps", 3, "PSUM"),
        ("zps", 2, "PSUM"),
    ):
        kw = {"space": space} if space == "PSUM" else {}
        pools[name] = ctx.enter_context(tc.tile_pool(name=name, bufs=bufs, **kw))
    consts_pool = ctx.enter_context(tc.tile_pool(name="consts", bufs=1))

    # ---- constants ----
    consts = {}
    ident_f = consts_pool.tile([P, P], F32)
    make_identity(nc, ident_f[:, :])
    ident8 = consts_pool.tile([P, P], F8)
    nc.vector.tensor_copy(ident8[:, :], ident_f[:, :])
    consts["ident8"] = ident8

    ipair = consts_pool.tile([P, 2, 2 * P], F8)
    nc.gpsimd.memset(ipair[:, :, :], 0.0)
    nc.vector.tensor_copy(ipair[:, 0, 0:P], ident8[:, :])
    nc.vector.tensor_copy(ipair[:, 1, P : 2 * P], ident8[:, :])
    consts["ipair"] = ipair

    ones8 = consts_pool.tile([P, 2, 8], F8)
    nc.gpsimd.memset(ones8[:, :, :], 1.0)
    consts["ones8"] = ones8

    dgp = consts_pool.tile([P, TP, 2, 2 * P], F8)
    nc.gpsimd.memset(dgp[:, :, :, :], 0.0)
    consts["dgp"] = dgp

    sg = _Stages(nc, pools, consts, x_ap, out_ap)
    B = B_PER_CORE

    # ---- prologue: batch 0 fully prepared, batch 1 loading ----
    sg.load(0)
    sg.alloc_eb(0)
    for t in range(T):
        sg.exp_tile(0, t)
    sg.stats(0)
    sg.dgp(0)
    sg.alloc_x8(0)
    for t in range(T):
        sg.cast_x8(0, t)
    sg.load(1)
    sg.alloc_x8(1)
    for t in range(T):
        sg.cast_x8(1, t)
    for k in range(T):
        sg.xtT(0, [k])
        sg.ptT(0, [k])
    for t in range(T):
        sg.cast_xl(0, t)

    # ---- steady state: PE order MM1(b), MM2(b), T(b+1). Windows per
    # engine: ACT est_f(b) | eb8(b+1) | pt-copies(b+1); DVE est_lo(b) |
    # rzi/out(b) | xt-copies(b+1) | xl(b+1); Pool est_hi(b) | x8(b+1)+dgp
    for b in range(B):
        sg.alloc_est(b)
        for j in range(T):
            sg.mm1_j(b, j)
        if b + 1 < B:
            sg.alloc_eb(b + 1)
            sg.xtT(b + 1, [0, 1])
        sg.mm2_products(b, 0)
        for i in range(1, T):
            sg.mm2_products(b, i)
            if b + 1 < B and i <= 6:
                sg.xtT(b + 1, [i + 1])
            sg.mm2_zout(b, i - 1)
            if b + 1 < B:
                sg.exp_tile(b + 1, i - 1)
        sg.mm2_zout(b, T - 1)
        if b + 1 < B:
            sg.exp_tile(b + 1, T - 1)
            sg.stats(b + 1)
            sg.dgp(b + 1)
            if b + 2 < B:
                sg.load(b + 2)
            sg.ptT(b + 1, list(range(T)))
            if b + 2 < B:
                sg.alloc_x8(b + 2)
                for t in range(T):
                    sg.cast_x8(b + 2, t)
            for t in range(T):
                sg.cast_xl(b + 1, t)


_CACHED = {}


def _build():
    if "nc" in _CACHED:
        return _CACHED["nc"]
    nc = bacc.Bacc(
        "TRN2",
        target_bir_lowering=False,
        debug=False,
        enable_asserts=False,
        num_devices=N_CORES,
    )
    x_ap = nc.dram_tensor("x", [B_PER_CORE, N, D], F32, kind="ExternalInput").ap()
    out_ap = nc.dram_tensor(
        "out", [B_PER_CORE, N, D], F32, kind="ExternalOutput"
    ).ap()
    with tile.TileContext(nc) as tc:
        with ExitStack() as ctx:
            build_kernel_body(ctx, tc, x_ap, out_ap)
    nc.compile()
    _CACHED["nc"] = nc
    return nc


LAST_EXEC_NS = None


def kernel(x: np.ndarray) -> np.ndarray:
    global LAST_EXEC_NS
    x = np.ascontiguousarray(np.asarray(x, dtype=np.float32))
    B = x.shape[0]
    assert B == N_CORES * B_PER_CORE and x.shape[1:] == (N, D)
    nc = _build()
    shards = x.reshape(N_CORES, B_PER_CORE, N, D)
    in_maps = [{"x": np.ascontiguousarray(shards[i])} for i in range(N_CORES)]
    trace = os.environ.get("KL_TRACE", "0") == "1"
    res = run_bass_kernel_spmd(
        nc, in_maps, core_ids=list(range(N_CORES)), trace=trace
    )
    LAST_EXEC_NS = res.exec_time_ns
    out = np.concatenate([r["out"] for r in res.results], axis=0)
    return out.astype(np.float32, copy=False)


# revision 24
# speedup vs baseline: 1.0014x; 1.0014x over previous
"""KL-attention kernel for Trainium2, 8-core data-parallel over batch.

Math (per batch b, x = [N=1024, D=1024] fp32):
  p = softmax(x, -1); logp = log_softmax(x, -1)
  S[i,j] = sum_d p[i,d] logp[j,d]   (attn = softmax(S, -1); the neg-entropy
                                     row offset cancels in the row softmax)
  Using sum_d p[i,d] = 1:  S[i,j] = (p @ x^T)[i,j] - logZ[j]
  out = softmax(S, -1) @ x

fp8 implementation (all PE work is float8e4 DoubleRow matmuls, K=256 per
pass, 0.5 cycles/row in the cost model):
  E8 = fp8(exp(x)), row-accumulated -> Z                       (ACT)
  pt8 = fp8( E8^T * diag(rz*SC) )   via DR matmul vs paired block-diag
  xt8 = x8^T                        via DR matmul vs paired identity
  ps_s[j,i] = SC * sum_d x8[j,d] p[i,d]                        (MM1, DR)
  est_f = exp(ps_s / SC)            (ACT, scale immediate)
  est_hi = fp8(est_f * rzc_j); est_lo = fp8(est_f*rzc_j - est_hi)
    where rzc = rz * e^C carries the -logZ_j column bias multiplicatively
  U = hi@x8 + hi@x_lo + lo@x8 (DR); z = (hi+lo)@ones (DR)      (MM2)
  out = U * (1/z)                    (ACT copy with per-partition scale)

The est/x residual splits keep fp8 quantization error ~5e-3 (validated in
numpy; single-fp8 would be ~3e-2). Batches are software-pipelined: batch
b+1's exp/casts interleave with batch b's MM1, its transposes run between
MM1(b) and MM2(b) on the PE, and its psum copies interleave with MM2(b)'s
output scaling, so every engine sees a dense FIFO.
"""

import os

import numpy as np

try:
    import concourse.bass as bass  # noqa: F401
except ImportError:
    import sys

    sys.path.insert(0, "/opt/trn_rl_repo")

from contextlib import ExitStack

import concourse.bass as bass
import concourse.mybir as mybir
import concourse.tile as tile
from concourse import bacc
from concourse.bass_utils import run_bass_kernel_spmd
from concourse.masks import make_identity

F32 = mybir.dt.float32
BF16 = mybir.dt.bfloat16
F8 = mybir.dt.float8e4
AF = mybir.ActivationFunctionType
DR = mybir.MatmulPerfMode.DoubleRow
ALU = mybir.AluOpType

N_CORES = 8
B_PER_CORE = 4
N = 1024
D = 1024
P = 128
T = N // P  # 8 row tiles
TP = T // 2  # 4 tile pairs (DoubleRow K=256)
SC = 512.0  # p scaling so fp8(p*SC) stays in normal range
C = 7.43  # ~log(N*E[e^x]) rescale for est into fp8 range

# engine split for psum->sbuf fp8 copy passes (xt8, pt8): tiles 0..a-1 on
# ACT, the rest on DVE. (Pool/GPSIMD cannot read PSUM.)
XT_ACT = 3
PT_ACT = 2


class _Stages:
    """Per-batch stage emitters over shared pools; call order = schedule."""

    def __init__(self, nc, pools, consts, x_ap, out_ap):
        self.nc = nc
        self.p = pools
        self.c = consts
        self.x_ap = x_ap
        self.out_ap = out_ap
        self.s = [dict() for _ in range(B_PER_CORE)]

    def load(self, b):
        st = self.s[b]
        st["xf"] = self.p["xf"].tile([P, T, D], F32, tag="xf", name="xf")
        for t in range(T):
            self.nc.sync.dma_start(
                st["xf"][:, t, :], self.x_ap[b, t * P : (t + 1) * P, :]
            )

    def alloc_eb(self, b):
        st = self.s[b]
        st["eb8"] = self.p["eb"].tile([P, T, D], F8, tag="eb", name="eb8")
        st["zs"] = self.p["st"].tile([P, T], F32, tag="zs", name="zs")

    def exp_tile(self, b, t):
        st = self.s[b]
        self.nc.scalar.activation(
            st["eb8"][:, t, :],
            st["xf"][:, t, :],
            AF.Exp,
            accum_out=st["zs"][:, t : t + 1],
        )

    def stats(self, b):
        st = self.s[b]
        st["rz"] = self.p["st"].tile([P, T], F32, tag="rz", name="rz")
        st["rzsc"] = self.p["st"].tile([P, T], F32, tag="rzsc", name="rzsc")
        st["rzc"] = self.p["st"].tile([P, T], F32, tag="rzc", name="rzc")
        self.nc.vector.reciprocal(st["rz"][:, :], st["zs"][:, :])
        self.nc.vector.tensor_scalar_mul(st["rzsc"][:, :], st["rz"][:, :], SC)
        self.nc.vector.tensor_scalar_mul(
            st["rzc"][:, :], st["rz"][:, :], float(np.exp(C))
        )

    def dgp(self, b):
        st = self.s[b]
        st["dgp"] = self.c["dgp"]  # shared zeroed tile; diag blocks rewritten
        dgp, rzsc = st["dgp"], st["rzsc"]
        for m in range(TP):
            self.nc.gpsimd.tensor_scalar_mul(
                dgp[:, m, 0, 0:P], self.c["ident8"][:, :], rzsc[:, 2 * m : 2 * m + 1]
            )
            self.nc.gpsimd.tensor_scalar_mul(
                dgp[:, m, 1, P : 2 * P],
                self.c["ident8"][:, :],
                rzsc[:, 2 * m + 1 : 2 * m + 2],
            )

    def alloc_x8(self, b):
        st = self.s[b]
        st["x8"] = self.p["x8"].tile([P, T, D], F8, tag="x8", name="x8")
        st["xl8"] = self.p["xl"].tile([P, T, D], F8, tag="xl", name="xl8")

    def cast_x8(self, b, t):
        st = self.s[b]
        self.nc.gpsimd.tensor_copy(st["x8"][:, t, :], st["xf"][:, t, :])

    def cast_xl(self, b, t):
        st = self.s[b]
        self.nc.vector.scalar_tensor_tensor(
            st["xl8"][:, t, :],
            st["xf"][:, t, :],
            1.0,
            st["x8"][:, t, :],
            ALU.mult,
            ALU.subtract,
        )

    def xtT(self, b, ks):
        nc, st = self.nc, self.s[b]
        if "xt8" not in st:
            st["xt8"] = self.p["xt"].tile([P, T, D], F8, tag="xt", name="xt8")
        for k in ks:
            ps_x = self.p["mmps"].tile([P, D], F32, tag="ps", name="ps")
            for m in range(TP):
                nc.tensor.matmul(
                    ps_x[:, m * 2 * P : (m + 1) * 2 * P],
                    st["x8"][:, 2 * m : 2 * m + 2, k * P : (k + 1) * P],
                    self.c["ipair"][:, :, :],
                    start=True,
                    stop=True,
                    perf_mode=DR,
                )
            if k < 3:
                nc.scalar.copy(st["xt8"][:, k, :], ps_x[:, :])
            else:
                nc.vector.tensor_copy(st["xt8"][:, k, :], ps_x[:, :])

    def ptT(self, b, ks):
        nc, st = self.nc, self.s[b]
        if "pt8" not in st:
            st["pt8"] = self.p["pt"].tile([P, T, D], F8, tag="pt", name="pt8")
        for k in ks:
            ps_p = self.p["mmps"].tile([P, D], F32, tag="ps", name="ps")
            for m in range(TP):
                nc.tensor.matmul(
                    ps_p[:, m * 2 * P : (m + 1) * 2 * P],
                    st["eb8"][:, 2 * m : 2 * m + 2, k * P : (k + 1) * P],
                    st["dgp"][:, m, :, :],
                    start=True,
                    stop=True,
                    perf_mode=DR,
                )
            nc.scalar.copy(st["pt8"][:, k, :], ps_p[:, :])

    def alloc_est(self, b):
        st = self.s[b]
        st["eh8"] = self.p["eh"].tile([P, T, D], F8, tag="eh", name="eh8")
        st["el8"] = self.p["el"].tile([P, T, D], F8, tag="el", name="el8")

    def mm1_j(self, b, j):
        nc, st = self.nc, self.s[b]
        ps_s = self.p["mmps"].tile([P, D], F32, tag="ps", name="ps")
        for dp in range(TP):
            for c in range(2):
                nc.tensor.matmul(
                    ps_s[:, c * 512 : (c + 1) * 512],
                    st["xt8"][:, 2 * dp : 2 * dp + 2, j * P : (j + 1) * P],
                    st["pt8"][:, 2 * dp : 2 * dp + 2, c * 512 : (c + 1) * 512],
                    start=(dp == 0),
                    stop=(dp == TP - 1),
                    perf_mode=DR,
                )
        est_f = self.p["ef"].tile([P, D], F32, tag="ef", name="est_f")
        nc.scalar.activation(est_f[:, :], ps_s[:, :], AF.Exp, scale=1.0 / SC)
        if j < 6:
            nc.gpsimd.tensor_scalar_mul(
                st["eh8"][:, j, :], est_f[:, :], st["rzc"][:, j : j + 1]
            )
        else:
            nc.scalar.mul(st["eh8"][:, j, :], est_f[:, :], st["rzc"][:, j : j + 1])
        nc.vector.scalar_tensor_tensor(
            st["el8"][:, j, :],
            est_f[:, :],
            st["rzc"][:, j : j + 1],
            st["eh8"][:, j, :],
            ALU.mult,
            ALU.subtract,
        )

    def mm2_products(self, b, i):
        nc, st = self.nc, self.s[b]
        st[f"ps_o{i}"] = ps_o = self.p["mmps"].tile([P, D], F32, tag="ps", name="ps")
        prods = ((st["eh8"], st["x8"]), (st["eh8"], st["xl8"]), (st["el8"], st["x8"]))
        # jp outer so early j-pairs of est can feed the PE before the whole
        # est split finishes (range-precise deps let MM2 crawl with MM1)
        idx = 0
        for jp in range(TP):
            for lhs, rhs in prods:
                for c in range(2):
                    nc.tensor.matmul(
                        ps_o[:, c * 512 : (c + 1) * 512],
                        lhs[:, 2 * jp : 2 * jp + 2, i * P : (i + 1) * P],
                        rhs[:, 2 * jp : 2 * jp + 2, c * 512 : (c + 1) * 512],
                        start=(idx == 0),
                        stop=(idx == 3 * TP - 1),
                        perf_mode=DR,
                    )
                idx += 1

    def mm2_zout(self, b, i):
        nc, st = self.nc, self.s[b]
        ps_o = st.pop(f"ps_o{i}")
        ps_z = self.p["zps"].tile([P, 8], F32, tag="psz", name="psz")
        for jp in range(TP):
            for li, lhs in ((0, st["eh8"]), (1, st["el8"])):
                nc.tensor.matmul(
                    ps_z[:, 0:8],
                    lhs[:, 2 * jp : 2 * jp + 2, i * P : (i + 1) * P],
                    self.c["ones8"][:, :, :],
                    start=(jp == 0 and li == 0),
                    stop=(jp == TP - 1 and li == 1),
                    perf_mode=DR,
                )
        rzi = self.p["st"].tile([P, 1], F32, tag="rzi", name="rzi")
        nc.vector.reciprocal(rzi[:, :], ps_z[:, 0:1])
        outf = self.p["of"].tile([P, D], F32, tag="of", name="outf")
        if i < 3:
            nc.scalar.mul(outf[:, :], ps_o[:, :], rzi[:, :])
        else:
            nc.vector.tensor_scalar_mul(outf[:, :], ps_o[:, :], rzi[:, :])
        nc.sync.dma_start(self.out_ap[b, i * P : (i + 1) * P, :], outf[:, :])


def build_kernel_body(ctx: ExitStack, tc: "tile.TileContext", x_ap, out_ap):
    nc = tc.nc

    pools = {}
    for name, bufs, space in (
        ("xf", 2, "SBUF"),
        ("eb", 2, "SBUF"),
        ("x8", 3, "SBUF"),
        ("xl", 2, "SBUF"),
        ("xt", 1, "SBUF"),
        ("pt", 1, "SBUF"),
        ("ef", 3, "SBUF"),
        ("eh", 1, "SBUF"),
        ("el", 1, "SBUF"),
        ("of", 4, "SBUF"),
        ("st", 2, "SBUF"),
        ("mmps", 3, "PSUM"),
        ("zps", 2, "PSUM"),
    ):
        kw = {"space": space} if space == "PSUM" else {}
        pools[name] = ctx.enter_context(tc.tile_pool(name=name, bufs=bufs, **kw))
    consts_pool = ctx.enter_context(tc.tile_pool(name="consts", bufs=1))

    # ---- constants ----
    consts = {}
    ident_f = consts_pool.tile([P, P], F32)
    make_identity(nc, ident_f[:, :])
    ident8 = consts_pool.tile([P, P], F8)
    nc.vector.tensor_copy(ident8[:, :], ident_f[:, :])
    consts["ident8"] = ident8

    ipair = consts_pool.tile([P, 2, 2 * P], F8)
    nc.gpsimd.memset(ipair[:, :, :], 0.0)
    nc.vector.tensor_copy(ipair[:, 0, 0:P], ident8[:, :])
    nc.vector.tensor_copy(ipair[:, 1, P : 2 * P], ident8[:, :])
    consts["ipair"] = ipair

    ones8 = consts_pool.tile([P, 2, 8], F8)
    nc.gpsimd.memset(ones8[:, :, :], 1.0)
    consts["ones8"] = ones8

    dgp = consts_pool.tile([P, TP, 2, 2 * P], F8)
    nc.gpsimd.memset(dgp[:, :, :, :], 0.0)
    consts["dgp"] = dgp

    sg = _Stages(nc, pools, consts, x_ap, out_ap)
    B = B_PER_CORE

    # ---- prologue: batch 0 fully prepared, batch 1 loading ----
    sg.load(0)
    sg.alloc_eb(0)
    for t in range(T):
        sg.exp_tile(0, t)
    sg.stats(0)
    sg.dgp(0)
    sg.alloc_x8(0)
    for t in range(T):
        sg.cast_x8(0, t)
    sg.load(1)
    sg.alloc_x8(1)
    for t in range(T):
        sg.cast_x8(1, t)
    for k in range(T):
        sg.xtT(0, [k])
        sg.ptT(0, [k])
    for t in range(T):
        sg.cast_xl(0, t)

    # ---- steady state: PE order MM1(b), MM2(b), T(b+1). Windows per
    # engine: ACT est_f(b) | eb8(b+1) | pt-copies(b+1); DVE est_lo(b) |
    # rzi/out(b) | xt-copies(b+1) | xl(b+1); Pool est_hi(b) | x8(b+1)+dgp
    for b in range(B):
        sg.alloc_est(b)
        for j in range(T):
            sg.mm1_j(b, j)
        if b + 1 < B:
            sg.alloc_eb(b + 1)
            sg.xtT(b + 1, [0, 1])
        sg.mm2_products(b, 0)
        for i in range(1, T):
            sg.mm2_products(b, i)
            if b + 1 < B and i <= 6:
                sg.xtT(b + 1, [i + 1])
            sg.mm2_zout(b, i - 1)
            if b + 1 < B:
                sg.exp_tile(b + 1, i - 1)
        sg.mm2_zout(b, T - 1)
        if b + 1 < B:
            sg.exp_tile(b + 1, T - 1)
            sg.stats(b + 1)
            sg.dgp(b + 1)
            if b + 2 < B:
                sg.load(b + 2)
            sg.ptT(b + 1, list(range(T)))
            if b + 2 < B:
                sg.alloc_x8(b + 2)
                for t in range(T):
                    sg.cast_x8(b + 2, t)
            for t in range(T):
                sg.cast_xl(b + 1, t)


_CACHED = {}


def _build():
    if "nc" in _CACHED:
        return _CACHED["nc"]
    nc = bacc.Bacc(
        "TRN2",
        target_bir_lowering=False,
        debug=False,
        enable_asserts=False,
        num_devices=N_CORES,
    )
    x_ap = nc.dram_tensor("x", [B_PER_CORE, N, D], F32, kind="ExternalInput").ap()
    out_ap = nc.dram_tensor(
        "out", [B_PER_CORE, N, D], F32, kind="ExternalOutput"
    ).ap()
    with tile.TileContext(nc) as tc:
        with ExitStack() as ctx:
            build_kernel_body(ctx, tc, x_ap, out_ap)
    nc.compile()
    _CACHED["nc"] = nc
    return nc


LAST_EXEC_NS = None


def kernel(x: np.ndarray) -> np.ndarray:
    global LAST_EXEC_NS
    x = np.ascontiguousarray(np.asarray(x, dtype=np.float32))
    B = x.shape[0]
    assert B == N_CORES * B_PER_CORE and x.shape[1:] == (N, D)
    nc = _build()
    shards = x.reshape(N_CORES, B_PER_CORE, N, D)
    in_maps = [{"x": np.ascontiguousarray(shards[i])} for i in range(N_CORES)]
    trace = os.environ.get("KL_TRACE", "0") == "1"
    res = run_bass_kernel_spmd(
        nc, in_maps, core_ids=list(range(N_CORES)), trace=trace
    )
    LAST_EXEC_NS = res.exec_time_ns
    out = np.concatenate([r["out"] for r in res.results], axis=0)
    return out.astype(np.float32, copy=False)


# revision 25
# speedup vs baseline: 1.0540x; 1.0526x over previous
"""KL-attention kernel for Trainium2, 8-core data-parallel over batch.

Math (per batch b, x = [N=1024, D=1024] fp32):
  p = softmax(x, -1); logp = log_softmax(x, -1)
  S[i,j] = sum_d p[i,d] logp[j,d]   (attn = softmax(S, -1); the neg-entropy
                                     row offset cancels in the row softmax)
  Using sum_d p[i,d] = 1:  S[i,j] = (p @ x^T)[i,j] - logZ[j]
  out = softmax(S, -1) @ x

fp8 implementation (all PE work is float8e4 DoubleRow matmuls, K=256 per
pass, 0.5 cycles/row in the cost model):
  E8 = fp8(exp(x)), row-accumulated -> Z                       (ACT)
  pt8 = fp8( E8^T * diag(rz*SC) )   via DR matmul vs paired block-diag
  xt8 = x8^T                        via DR matmul vs paired identity
  ps_s[j,i] = SC * sum_d x8[j,d] p[i,d]                        (MM1, DR)
  est_f = exp(ps_s / SC)            (ACT, scale immediate)
  est_hi = fp8(est_f * rzc_j); est_lo = fp8(est_f*rzc_j - est_hi)
    where rzc = rz * e^C carries the -logZ_j column bias multiplicatively
  U = hi@x8 + hi@x_lo + lo@x8 (DR); z = (hi+lo)@ones (DR)      (MM2)
  out = U * (1/z)                    (ACT copy with per-partition scale)

The est/x residual splits keep fp8 quantization error ~5e-3 (validated in
numpy; single-fp8 would be ~3e-2). Batches are software-pipelined: batch
b+1's exp/casts interleave with batch b's MM1, its transposes run between
MM1(b) and MM2(b) on the PE, and its psum copies interleave with MM2(b)'s
output scaling, so every engine sees a dense FIFO.
"""

import os

import numpy as np

try:
    import concourse.bass as bass  # noqa: F401
except ImportError:
    import sys

    sys.path.insert(0, "/opt/trn_rl_repo")

from contextlib import ExitStack

import concourse.bass as bass
import concourse.mybir as mybir
import concourse.tile as tile
from concourse import bacc
from concourse.bass_utils import run_bass_kernel_spmd
from concourse.masks import make_identity

F32 = mybir.dt.float32
BF16 = mybir.dt.bfloat16
F8 = mybir.dt.float8e4
AF = mybir.ActivationFunctionType
DR = mybir.MatmulPerfMode.DoubleRow
ALU = mybir.AluOpType

N_CORES = 8
B_PER_CORE = 4
N = 1024
D = 1024
P = 128
T = N // P  # 8 row tiles
TP = T // 2  # 4 tile pairs (DoubleRow K=256)
SC = 512.0  # p scaling so fp8(p*SC) stays in normal range
C = 7.43  # ~log(N*E[e^x]) rescale for est into fp8 range

# engine split for psum->sbuf fp8 copy passes (xt8, pt8): tiles 0..a-1 on
# ACT, the rest on DVE. (Pool/GPSIMD cannot read PSUM.)
XT_ACT = 3
PT_ACT = 2


class _Stages:
    """Per-batch stage emitters over shared pools; call order = schedule."""

    def __init__(self, nc, pools, consts, x_ap, out_ap):
        self.nc = nc
        self.p = pools
        self.c = consts
        self.x_ap = x_ap
        self.out_ap = out_ap
        self.s = [dict() for _ in range(B_PER_CORE)]

    def load(self, b):
        st = self.s[b]
        st["xf"] = self.p["xf"].tile([P, T, D], F32, tag="xf", name="xf")
        for t in range(T):
            self.nc.sync.dma_start(
                st["xf"][:, t, :], self.x_ap[b, t * P : (t + 1) * P, :]
            )

    def alloc_eb(self, b):
        st = self.s[b]
        st["eb8"] = self.p["eb"].tile([P, T, D], F8, tag="eb", name="eb8")
        st["zs"] = self.p["st"].tile([P, T], F32, tag="zs", name="zs")

    def exp_tile(self, b, t):
        st = self.s[b]
        self.nc.scalar.activation(
            st["eb8"][:, t, :],
            st["xf"][:, t, :],
            AF.Exp,
            accum_out=st["zs"][:, t : t + 1],
        )

    def stats(self, b):
        st = self.s[b]
        st["rz"] = self.p["st"].tile([P, T], F32, tag="rz", name="rz")
        st["rzsc"] = self.p["st"].tile([P, T], F32, tag="rzsc", name="rzsc")
        st["rzc"] = self.p["st"].tile([P, T], F32, tag="rzc", name="rzc")
        self.nc.vector.reciprocal(st["rz"][:, :], st["zs"][:, :])
        self.nc.vector.tensor_scalar_mul(st["rzsc"][:, :], st["rz"][:, :], SC)
        self.nc.vector.tensor_scalar_mul(
            st["rzc"][:, :], st["rz"][:, :], float(np.exp(C))
        )

    def dgp(self, b):
        st = self.s[b]
        st["dgp"] = self.c["dgp"]  # shared zeroed tile; diag blocks rewritten
        dgp, rzsc = st["dgp"], st["rzsc"]
        for m in range(TP):
            self.nc.gpsimd.tensor_scalar_mul(
                dgp[:, m, 0, 0:P], self.c["ident8"][:, :], rzsc[:, 2 * m : 2 * m + 1]
            )
            self.nc.gpsimd.tensor_scalar_mul(
                dgp[:, m, 1, P : 2 * P],
                self.c["ident8"][:, :],
                rzsc[:, 2 * m + 1 : 2 * m + 2],
            )

    def alloc_x8(self, b):
        st = self.s[b]
        st["x8"] = self.p["x8"].tile([P, T, D], F8, tag="x8", name="x8")
        st["xl8"] = self.p["xl"].tile([P, T, D], F8, tag="xl", name="xl8")

    def cast_x8(self, b, t):
        st = self.s[b]
        self.nc.gpsimd.tensor_copy(st["x8"][:, t, :], st["xf"][:, t, :])

    def cast_xl(self, b, t):
        st = self.s[b]
        self.nc.vector.scalar_tensor_tensor(
            st["xl8"][:, t, :],
            st["xf"][:, t, :],
            1.0,
            st["x8"][:, t, :],
            ALU.mult,
            ALU.subtract,
        )

    def transposes(self, b):
        nc, st = self.nc, self.s[b]
        st["xt8"] = self.p["xt"].tile([P, T, D], F8, tag="xt", name="xt8")
        st["pt8"] = self.p["pt"].tile([P, T, D], F8, tag="pt", name="pt8")
        # interleave xt/pt k-groups so the xt copies (DVE) and pt copies
        # (ACT) drain in parallel instead of serializing the T window
        for k in range(T):
            ps_x = self.p["mmps"].tile([P, D], F32, tag="ps", name="ps")
            for m in range(TP):
                nc.tensor.matmul(
                    ps_x[:, m * 2 * P : (m + 1) * 2 * P],
                    st["x8"][:, 2 * m : 2 * m + 2, k * P : (k + 1) * P],
                    self.c["ipair"][:, :, :],
                    start=True,
                    stop=True,
                    perf_mode=DR,
                )
            if k < 3:
                nc.scalar.copy(st["xt8"][:, k, :], ps_x[:, :])
            else:
                nc.vector.tensor_copy(st["xt8"][:, k, :], ps_x[:, :])
            ps_p = self.p["mmps"].tile([P, D], F32, tag="ps", name="ps")
            for m in range(TP):
                nc.tensor.matmul(
                    ps_p[:, m * 2 * P : (m + 1) * 2 * P],
                    st["eb8"][:, 2 * m : 2 * m + 2, k * P : (k + 1) * P],
                    st["dgp"][:, m, :, :],
                    start=True,
                    stop=True,
                    perf_mode=DR,
                )
            nc.scalar.copy(st["pt8"][:, k, :], ps_p[:, :])

    def alloc_est(self, b):
        st = self.s[b]
        st["eh8"] = self.p["eh"].tile([P, T, D], F8, tag="eh", name="eh8")
        st["el8"] = self.p["el"].tile([P, T, D], F8, tag="el", name="el8")

    def mm1_j(self, b, j):
        nc, st = self.nc, self.s[b]
        ps_s = self.p["mmps"].tile([P, D], F32, tag="ps", name="ps")
        for dp in range(TP):
            for c in range(2):
                nc.tensor.matmul(
                    ps_s[:, c * 512 : (c + 1) * 512],
                    st["xt8"][:, 2 * dp : 2 * dp + 2, j * P : (j + 1) * P],
                    st["pt8"][:, 2 * dp : 2 * dp + 2, c * 512 : (c + 1) * 512],
                    start=(dp == 0),
                    stop=(dp == TP - 1),
                    perf_mode=DR,
                )
        est_f = self.p["ef"].tile([P, D], F32, tag="ef", name="est_f")
        nc.scalar.activation(est_f[:, :], ps_s[:, :], AF.Exp, scale=1.0 / SC)
        if j < 6:
            nc.gpsimd.tensor_scalar_mul(
                st["eh8"][:, j, :], est_f[:, :], st["rzc"][:, j : j + 1]
            )
        else:
            nc.scalar.mul(st["eh8"][:, j, :], est_f[:, :], st["rzc"][:, j : j + 1])
        nc.vector.scalar_tensor_tensor(
            st["el8"][:, j, :],
            est_f[:, :],
            st["rzc"][:, j : j + 1],
            st["eh8"][:, j, :],
            ALU.mult,
            ALU.subtract,
        )

    def mm2_products(self, b, i):
        nc, st = self.nc, self.s[b]
        st[f"ps_o{i}"] = ps_o = self.p["mmps"].tile([P, D], F32, tag="ps", name="ps")
        prods = ((st["eh8"], st["x8"]), (st["eh8"], st["xl8"]), (st["el8"], st["x8"]))
        # jp outer so early j-pairs of est can feed the PE before the whole
        # est split finishes (range-precise deps let MM2 crawl with MM1)
        idx = 0
        for jp in range(TP):
            for lhs, rhs in prods:
                for c in range(2):
                    nc.tensor.matmul(
                        ps_o[:, c * 512 : (c + 1) * 512],
                        lhs[:, 2 * jp : 2 * jp + 2, i * P : (i + 1) * P],
                        rhs[:, 2 * jp : 2 * jp + 2, c * 512 : (c + 1) * 512],
                        start=(idx == 0),
                        stop=(idx == 3 * TP - 1),
                        perf_mode=DR,
                    )
                idx += 1

    def mm2_zout(self, b, i):
        nc, st = self.nc, self.s[b]
        ps_o = st.pop(f"ps_o{i}")
        ps_z = self.p["zps"].tile([P, 8], F32, tag="psz", name="psz")
        for jp in range(TP):
            for li, lhs in ((0, st["eh8"]), (1, st["el8"])):
                nc.tensor.matmul(
                    ps_z[:, 0:8],
                    lhs[:, 2 * jp : 2 * jp + 2, i * P : (i + 1) * P],
                    self.c["ones8"][:, :, :],
                    start=(jp == 0 and li == 0),
                    stop=(jp == TP - 1 and li == 1),
                    perf_mode=DR,
                )
        rzi = self.p["st"].tile([P, 1], F32, tag="rzi", name="rzi")
        nc.vector.reciprocal(rzi[:, :], ps_z[:, 0:1])
        outf = self.p["of"].tile([P, D], F32, tag="of", name="outf")
        if i < 3:
            nc.scalar.mul(outf[:, :], ps_o[:, :], rzi[:, :])
        else:
            nc.vector.tensor_scalar_mul(outf[:, :], ps_o[:, :], rzi[:, :])
        nc.sync.dma_start(self.out_ap[b, i * P : (i + 1) * P, :], outf[:, :])


def build_kernel_body(ctx: ExitStack, tc: "tile.TileContext", x_ap, out_ap):
    nc = tc.nc

    pools = {}
    for name, bufs, space in (
        ("xf", 2, "SBUF"),
        ("eb", 2, "SBUF"),
        ("x8", 2, "SBUF"),
        ("xl", 2, "SBUF"),
        ("xt", 1, "SBUF"),
        ("pt", 1, "SBUF"),
        ("ef", 3, "SBUF"),
        ("eh", 1, "SBUF"),
        ("el", 1, "SBUF"),
        ("of", 4, "SBUF"),
        ("st", 2, "SBUF"),
        ("mmps", 3, "PSUM"),
        ("zps", 2, "PSUM"),
    ):
        kw = {"space": space} if space == "PSUM" else {}
        pools[name] = ctx.enter_context(tc.tile_pool(name=name, bufs=bufs, **kw))
    consts_pool = ctx.enter_context(tc.tile_pool(name="consts", bufs=1))

    # ---- constants ----
    consts = {}
    ident_f = consts_pool.tile([P, P], F32)
    make_identity(nc, ident_f[:, :])
    ident8 = consts_pool.tile([P, P], F8)
    nc.vector.tensor_copy(ident8[:, :], ident_f[:, :])
    consts["ident8"] = ident8

    ipair = consts_pool.tile([P, 2, 2 * P], F8)
    nc.gpsimd.memset(ipair[:, :, :], 0.0)
    nc.vector.tensor_copy(ipair[:, 0, 0:P], ident8[:, :])
    nc.vector.tensor_copy(ipair[:, 1, P : 2 * P], ident8[:, :])
    consts["ipair"] = ipair

    ones8 = consts_pool.tile([P, 2, 8], F8)
    nc.gpsimd.memset(ones8[:, :, :], 1.0)
    consts["ones8"] = ones8

    dgp = consts_pool.tile([P, TP, 2, 2 * P], F8)
    nc.gpsimd.memset(dgp[:, :, :, :], 0.0)
    consts["dgp"] = dgp

    sg = _Stages(nc, pools, consts, x_ap, out_ap)
    B = B_PER_CORE

    # ---- prologue: batch 0 fully prepared, batch 1 loading ----
    sg.load(0)
    sg.alloc_eb(0)
    for t in range(T):
        sg.exp_tile(0, t)
    sg.stats(0)
    sg.dgp(0)
    sg.alloc_x8(0)
    for t in range(T):
        sg.cast_x8(0, t)
    sg.load(1)
    sg.transposes(0)
    for t in range(T):
        sg.cast_xl(0, t)

    # ---- steady state: PE order MM1(b), MM2(b), T(b+1). Windows per
    # engine: ACT est_f(b) | eb8(b+1) | pt-copies(b+1); DVE est_lo(b) |
    # rzi/out(b) | xt-copies(b+1) | xl(b+1); Pool est_hi(b) | x8(b+1)+dgp
    for b in range(B):
        sg.alloc_est(b)
        for j in range(T):
            sg.mm1_j(b, j)
        sg.mm2_products(b, 0)
        for i in range(1, T):
            sg.mm2_products(b, i)
            sg.mm2_zout(b, i - 1)
        sg.mm2_zout(b, T - 1)
        if b + 1 < B:
            sg.alloc_eb(b + 1)
            sg.alloc_x8(b + 1)
            for t in range(T):
                sg.exp_tile(b + 1, t)
            for t in range(T):
                sg.cast_x8(b + 1, t)
            sg.stats(b + 1)
            sg.dgp(b + 1)
            if b + 2 < B:
                sg.load(b + 2)
            sg.transposes(b + 1)
            for t in range(T):
                sg.cast_xl(b + 1, t)


_CACHED = {}


def _build():
    if "nc" in _CACHED:
        return _CACHED["nc"]
    nc = bacc.Bacc(
        "TRN2",
        target_bir_lowering=False,
        debug=False,
        enable_asserts=False,
        num_devices=N_CORES,
    )
    x_ap = nc.dram_tensor("x", [B_PER_CORE, N, D], F32, kind="ExternalInput").ap()
    out_ap = nc.dram_tensor(
        "out", [B_PER_CORE, N, D], F32, kind="ExternalOutput"
    ).ap()
    with tile.TileContext(nc) as tc:
        with ExitStack() as ctx:
            build_kernel_body(ctx, tc, x_ap, out_ap)
    nc.compile()
    _CACHED["nc"] = nc
    return nc


LAST_EXEC_NS = None


def kernel(x: np.ndarray) -> np.ndarray:
    global LAST_EXEC_NS
    x = np.ascontiguousarray(np.asarray(x, dtype=np.float32))
    B = x.shape[0]
    assert B == N_CORES * B_PER_CORE and x.shape[1:] == (N, D)
    nc = _build()
    shards = x.reshape(N_CORES, B_PER_CORE, N, D)
    in_maps = [{"x": np.ascontiguousarray(shards[i])} for i in range(N_CORES)]
    trace = os.environ.get("KL_TRACE", "0") == "1"
    res = run_bass_kernel_spmd(
        nc, in_maps, core_ids=list(range(N_CORES)), trace=trace
    )
    LAST_EXEC_NS = res.exec_time_ns
    out = np.concatenate([r["out"] for r in res.results], axis=0)
    return out.astype(np.float32, copy=False)


# revision 26
# speedup vs baseline: 1.0625x; 1.0081x over previous
"""KL-attention kernel for Trainium2, 8-core data-parallel over batch.

Math (per batch b, x = [N=1024, D=1024] fp32):
  p = softmax(x, -1); logp = log_softmax(x, -1)
  S[i,j] = sum_d p[i,d] logp[j,d]   (attn = softmax(S, -1); the neg-entropy
                                     row offset cancels in the row softmax)
  Using sum_d p[i,d] = 1:  S[i,j] = (p @ x^T)[i,j] - logZ[j]
  out = softmax(S, -1) @ x

fp8 implementation (all PE work is float8e4 DoubleRow matmuls, K=256 per
pass, 0.5 cycles/row in the cost model):
  E8 = fp8(exp(x)), row-accumulated -> Z                       (ACT)
  pt8 = fp8( E8^T * diag(rz*SC) )   via DR matmul vs paired block-diag
  xt8 = x8^T                        via DR matmul vs paired identity
  ps_s[j,i] = SC * sum_d x8[j,d] p[i,d]                        (MM1, DR)
  est_f = exp(ps_s / SC)            (ACT, scale immediate)
  est_hi = fp8(est_f * rzc_j); est_lo = fp8(est_f*rzc_j - est_hi)
    where rzc = rz * e^C carries the -logZ_j column bias multiplicatively
  U = hi@x8 + hi@x_lo + lo@x8 (DR); z = (hi+lo)@ones (DR)      (MM2)
  out = U * (1/z)                    (ACT copy with per-partition scale)

The est/x residual splits keep fp8 quantization error ~5e-3 (validated in
numpy; single-fp8 would be ~3e-2). Batches are software-pipelined: batch
b+1's exp/casts interleave with batch b's MM1, its transposes run between
MM1(b) and MM2(b) on the PE, and its psum copies interleave with MM2(b)'s
output scaling, so every engine sees a dense FIFO.
"""

import os

import numpy as np

try:
    import concourse.bass as bass  # noqa: F401
except ImportError:
    import sys

    sys.path.insert(0, "/opt/trn_rl_repo")

from contextlib import ExitStack

import concourse.bass as bass
import concourse.mybir as mybir
import concourse.tile as tile
from concourse import bacc
from concourse.bass_utils import run_bass_kernel_spmd
from concourse.masks import make_identity

F32 = mybir.dt.float32
BF16 = mybir.dt.bfloat16
F8 = mybir.dt.float8e4
AF = mybir.ActivationFunctionType
DR = mybir.MatmulPerfMode.DoubleRow
ALU = mybir.AluOpType

N_CORES = 8
B_PER_CORE = 4
N = 1024
D = 1024
P = 128
T = N // P  # 8 row tiles
TP = T // 2  # 4 tile pairs (DoubleRow K=256)
SC = 512.0  # p scaling so fp8(p*SC) stays in normal range
C = 7.43  # ~log(N*E[e^x]) rescale for est into fp8 range

# engine split for psum->sbuf fp8 copy passes (xt8, pt8): tiles 0..a-1 on
# ACT, the rest on DVE. (Pool/GPSIMD cannot read PSUM.)
XT_ACT = 3
PT_ACT = 2


class _Stages:
    """Per-batch stage emitters over shared pools; call order = schedule."""

    def __init__(self, nc, pools, consts, x_ap, out_ap):
        self.nc = nc
        self.p = pools
        self.c = consts
        self.x_ap = x_ap
        self.out_ap = out_ap
        self.s = [dict() for _ in range(B_PER_CORE)]

    def load(self, b):
        st = self.s[b]
        st["xf"] = self.p["xf"].tile([P, T, D], F32, tag="xf", name="xf")
        for t in range(T):
            self.nc.sync.dma_start(
                st["xf"][:, t, :], self.x_ap[b, t * P : (t + 1) * P, :]
            )

    def alloc_eb(self, b):
        st = self.s[b]
        st["eb8"] = self.p["eb"].tile([P, T, D], F8, tag="eb", name="eb8")
        st["zs"] = self.p["st"].tile([P, T], F32, tag="zs", name="zs")

    def exp_tile(self, b, t):
        st = self.s[b]
        self.nc.scalar.activation(
            st["eb8"][:, t, :],
            st["xf"][:, t, :],
            AF.Exp,
            accum_out=st["zs"][:, t : t + 1],
        )

    def stats(self, b):
        st = self.s[b]
        st["rz"] = self.p["st"].tile([P, T], F32, tag="rz", name="rz")
        st["rzsc"] = self.p["st"].tile([P, T], F32, tag="rzsc", name="rzsc")
        st["rzc"] = self.p["st"].tile([P, T], F32, tag="rzc", name="rzc")
        self.nc.vector.reciprocal(st["rz"][:, :], st["zs"][:, :])
        self.nc.vector.tensor_scalar_mul(st["rzsc"][:, :], st["rz"][:, :], SC)
        self.nc.vector.tensor_scalar_mul(
            st["rzc"][:, :], st["rz"][:, :], float(np.exp(C))
        )

    def dgp(self, b):
        st = self.s[b]
        st["dgp"] = self.c["dgp"]  # shared zeroed tile; diag blocks rewritten
        dgp, rzsc = st["dgp"], st["rzsc"]
        for m in range(TP):
            self.nc.gpsimd.tensor_scalar_mul(
                dgp[:, m, 0, 0:P], self.c["ident8"][:, :], rzsc[:, 2 * m : 2 * m + 1]
            )
            self.nc.gpsimd.tensor_scalar_mul(
                dgp[:, m, 1, P : 2 * P],
                self.c["ident8"][:, :],
                rzsc[:, 2 * m + 1 : 2 * m + 2],
            )

    def alloc_x8(self, b):
        st = self.s[b]
        st["x8"] = self.p["x8"].tile([P, T, D], F8, tag="x8", name="x8")
        st["xl8"] = self.p["xl"].tile([P, T, D], F8, tag="xl", name="xl8")

    def cast_x8(self, b, t):
        st = self.s[b]
        self.nc.gpsimd.tensor_copy(st["x8"][:, t, :], st["xf"][:, t, :])

    def cast_xl(self, b, t):
        st = self.s[b]
        self.nc.vector.scalar_tensor_tensor(
            st["xl8"][:, t, :],
            st["xf"][:, t, :],
            1.0,
            st["x8"][:, t, :],
            ALU.mult,
            ALU.subtract,
        )

    def transposes(self, b):
        nc, st = self.nc, self.s[b]
        st["xt8"] = self.p["xt"].tile([P, T, D], F8, tag="xt", name="xt8")
        st["pt8"] = self.p["pt"].tile([P, T, D], F8, tag="pt", name="pt8")
        # interleave xt/pt k-groups so the xt copies (DVE) and pt copies
        # (ACT) drain in parallel instead of serializing the T window
        for k in range(T):
            ps_x = self.p["mmps"].tile([P, D], F32, tag="ps", name="ps")
            for m in range(TP):
                nc.tensor.matmul(
                    ps_x[:, m * 2 * P : (m + 1) * 2 * P],
                    st["x8"][:, 2 * m : 2 * m + 2, k * P : (k + 1) * P],
                    self.c["ipair"][:, :, :],
                    start=True,
                    stop=True,
                    perf_mode=DR,
                )
            if k < 3:
                nc.scalar.copy(st["xt8"][:, k, :], ps_x[:, :])
            else:
                nc.vector.tensor_copy(st["xt8"][:, k, :], ps_x[:, :])
            ps_p = self.p["mmps"].tile([P, D], F32, tag="ps", name="ps")
            for m in range(TP):
                nc.tensor.matmul(
                    ps_p[:, m * 2 * P : (m + 1) * 2 * P],
                    st["eb8"][:, 2 * m : 2 * m + 2, k * P : (k + 1) * P],
                    st["dgp"][:, m, :, :],
                    start=True,
                    stop=True,
                    perf_mode=DR,
                )
            nc.scalar.copy(st["pt8"][:, k, :], ps_p[:, :])

    def alloc_est(self, b):
        st = self.s[b]
        st["eh8"] = self.p["eh"].tile([P, T, D], F8, tag="eh", name="eh8")
        st["el8"] = self.p["el"].tile([P, T, D], F8, tag="el", name="el8")

    def mm1_j(self, b, j):
        nc, st = self.nc, self.s[b]
        ps_s = self.p["mmps"].tile([P, D], F32, tag="ps", name="ps")
        for dp in range(TP):
            for c in range(2):
                nc.tensor.matmul(
                    ps_s[:, c * 512 : (c + 1) * 512],
                    st["xt8"][:, 2 * dp : 2 * dp + 2, j * P : (j + 1) * P],
                    st["pt8"][:, 2 * dp : 2 * dp + 2, c * 512 : (c + 1) * 512],
                    start=(dp == 0),
                    stop=(dp == TP - 1),
                    perf_mode=DR,
                )
        est_f = self.p["ef"].tile([P, D], F32, tag="ef", name="est_f")
        nc.scalar.activation(est_f[:, :], ps_s[:, :], AF.Exp, scale=1.0 / SC)
        if j < 6:
            nc.gpsimd.tensor_scalar_mul(
                st["eh8"][:, j, :], est_f[:, :], st["rzc"][:, j : j + 1]
            )
        else:
            nc.scalar.mul(st["eh8"][:, j, :], est_f[:, :], st["rzc"][:, j : j + 1])
        nc.vector.scalar_tensor_tensor(
            st["el8"][:, j, :],
            est_f[:, :],
            st["rzc"][:, j : j + 1],
            st["eh8"][:, j, :],
            ALU.mult,
            ALU.subtract,
        )

    def mm2_products(self, b, i):
        nc, st = self.nc, self.s[b]
        st[f"ps_o{i}"] = ps_o = self.p["mmps"].tile([P, D], F32, tag="ps", name="ps")
        prods = ((st["eh8"], st["x8"]), (st["eh8"], st["xl8"]), (st["el8"], st["x8"]))
        # jp outer so early j-pairs of est can feed the PE before the whole
        # est split finishes (range-precise deps let MM2 crawl with MM1)
        idx = 0
        for jp in range(TP):
            for lhs, rhs in prods:
                for c in range(2):
                    nc.tensor.matmul(
                        ps_o[:, c * 512 : (c + 1) * 512],
                        lhs[:, 2 * jp : 2 * jp + 2, i * P : (i + 1) * P],
                        rhs[:, 2 * jp : 2 * jp + 2, c * 512 : (c + 1) * 512],
                        start=(idx == 0),
                        stop=(idx == 3 * TP - 1),
                        perf_mode=DR,
                    )
                idx += 1

    def mm2_zout(self, b, i):
        nc, st = self.nc, self.s[b]
        ps_o = st.pop(f"ps_o{i}")
        ps_z = self.p["zps"].tile([P, 8], F32, tag="psz", name="psz")
        for jp in range(TP):
            for li, lhs in ((0, st["eh8"]), (1, st["el8"])):
                nc.tensor.matmul(
                    ps_z[:, 0:8],
                    lhs[:, 2 * jp : 2 * jp + 2, i * P : (i + 1) * P],
                    self.c["ones8"][:, :, :],
                    start=(jp == 0 and li == 0),
                    stop=(jp == TP - 1 and li == 1),
                    perf_mode=DR,
                )
        rzi = self.p["st"].tile([P, 1], F32, tag="rzi", name="rzi")
        nc.vector.reciprocal(rzi[:, :], ps_z[:, 0:1])
        outf = self.p["of"].tile([P, D], F32, tag="of", name="outf")
        if i < 3:
            nc.scalar.mul(outf[:, :], ps_o[:, :], rzi[:, :])
        else:
            nc.vector.tensor_scalar_mul(outf[:, :], ps_o[:, :], rzi[:, :])
        nc.sync.dma_start(self.out_ap[b, i * P : (i + 1) * P, :], outf[:, :])


def build_kernel_body(ctx: ExitStack, tc: "tile.TileContext", x_ap, out_ap):
    nc = tc.nc

    pools = {}
    for name, bufs, space in (
        ("xf", 2, "SBUF"),
        ("eb", 2, "SBUF"),
        ("x8", 2, "SBUF"),
        ("xl", 2, "SBUF"),
        ("xt", 1, "SBUF"),
        ("pt", 1, "SBUF"),
        ("ef", 4, "SBUF"),
        ("eh", 1, "SBUF"),
        ("el", 1, "SBUF"),
        ("of", 4, "SBUF"),
        ("st", 3, "SBUF"),
        ("mmps", 3, "PSUM"),
        ("zps", 2, "PSUM"),
    ):
        kw = {"space": space} if space == "PSUM" else {}
        pools[name] = ctx.enter_context(tc.tile_pool(name=name, bufs=bufs, **kw))
    consts_pool = ctx.enter_context(tc.tile_pool(name="consts", bufs=1))

    # ---- constants ----
    consts = {}
    ident_f = consts_pool.tile([P, P], F32)
    make_identity(nc, ident_f[:, :])
    ident8 = consts_pool.tile([P, P], F8)
    nc.vector.tensor_copy(ident8[:, :], ident_f[:, :])
    consts["ident8"] = ident8

    ipair = consts_pool.tile([P, 2, 2 * P], F8)
    nc.gpsimd.memset(ipair[:, :, :], 0.0)
    nc.vector.tensor_copy(ipair[:, 0, 0:P], ident8[:, :])
    nc.vector.tensor_copy(ipair[:, 1, P : 2 * P], ident8[:, :])
    consts["ipair"] = ipair

    ones8 = consts_pool.tile([P, 2, 8], F8)
    nc.gpsimd.memset(ones8[:, :, :], 1.0)
    consts["ones8"] = ones8

    dgp = consts_pool.tile([P, TP, 2, 2 * P], F8)
    nc.gpsimd.memset(dgp[:, :, :, :], 0.0)
    consts["dgp"] = dgp

    sg = _Stages(nc, pools, consts, x_ap, out_ap)
    B = B_PER_CORE

    # ---- prologue: batch 0 fully prepared, batch 1 loading ----
    sg.load(0)
    sg.alloc_eb(0)
    for t in range(T):
        sg.exp_tile(0, t)
    sg.stats(0)
    sg.dgp(0)
    sg.alloc_x8(0)
    for t in range(T):
        sg.cast_x8(0, t)
    sg.load(1)
    sg.transposes(0)
    for t in range(T):
        sg.cast_xl(0, t)

    # ---- steady state: PE order MM1(b), MM2(b), T(b+1). Windows per
    # engine: ACT est_f(b) | eb8(b+1) | pt-copies(b+1); DVE est_lo(b) |
    # rzi/out(b) | xt-copies(b+1) | xl(b+1); Pool est_hi(b) | x8(b+1)+dgp
    for b in range(B):
        sg.alloc_est(b)
        for j in range(T):
            sg.mm1_j(b, j)
        sg.mm2_products(b, 0)
        for i in range(1, T):
            sg.mm2_products(b, i)
            sg.mm2_zout(b, i - 1)
        sg.mm2_zout(b, T - 1)
        if b + 1 < B:
            sg.alloc_eb(b + 1)
            sg.alloc_x8(b + 1)
            for t in range(T):
                sg.exp_tile(b + 1, t)
            for t in range(T):
                sg.cast_x8(b + 1, t)
            sg.stats(b + 1)
            sg.dgp(b + 1)
            if b + 2 < B:
                sg.load(b + 2)
            sg.transposes(b + 1)
            for t in range(T):
                sg.cast_xl(b + 1, t)


_CACHED = {}


def _build():
    if "nc" in _CACHED:
        return _CACHED["nc"]
    nc = bacc.Bacc(
        "TRN2",
        target_bir_lowering=False,
        debug=False,
        enable_asserts=False,
        num_devices=N_CORES,
    )
    x_ap = nc.dram_tensor("x", [B_PER_CORE, N, D], F32, kind="ExternalInput").ap()
    out_ap = nc.dram_tensor(
        "out", [B_PER_CORE, N, D], F32, kind="ExternalOutput"
    ).ap()
    with tile.TileContext(nc) as tc:
        with ExitStack() as ctx:
            build_kernel_body(ctx, tc, x_ap, out_ap)
    nc.compile()
    _CACHED["nc"] = nc
    return nc


LAST_EXEC_NS = None


def kernel(x: np.ndarray) -> np.ndarray:
    global LAST_EXEC_NS
    x = np.ascontiguousarray(np.asarray(x, dtype=np.float32))
    B = x.shape[0]
    assert B == N_CORES * B_PER_CORE and x.shape[1:] == (N, D)
    nc = _build()
    shards = x.reshape(N_CORES, B_PER_CORE, N, D)
    in_maps = [{"x": np.ascontiguousarray(shards[i])} for i in range(N_CORES)]
    trace = os.environ.get("KL_TRACE", "0") == "1"
    res = run_bass_kernel_spmd(
        nc, in_maps, core_ids=list(range(N_CORES)), trace=trace
    )
    LAST_EXEC_NS = res.exec_time_ns
    out = np.concatenate([r["out"] for r in res.results], axis=0)
    return out.astype(np.float32, copy=False)
